# revision 16
# baseline (speedup 1.0000x reference)
"""Trainium2 Bass kernel for nn_AttentionLayer (dense transformer layer).

Reference computation (per batch b):
    q = x @ wq + bq ; k = x @ wk + bk ; v = x @ wv + bv
    scores = q @ k.T              (no scaling, no mask)
    probs  = softmax(scores, -1)
    attn   = probs @ v
    e      = LN1(x + attn) @ w0 + b0
    h      = LN2(lrelu(e @ w1 + b1))
    logits = h @ w2 + b2
    out    = LN3(lrelu(logits + e))

Sharding: data-parallel over batch. B=8 batches -> 8 NeuronCores, one batch
per core, weights replicated.  No collectives.

Per-core schedule (S=2048, D=1024, H=2048, P=128):
  Phase A: x -> xT (PE transpose, full [D,S] resident); weights streamed once
           as fp32r slabs via gpsimd casting DMA; kT -> DRAM scratch,
           qT -> DRAM scratch, v -> resident SBUF.
  Phase B: kT -> SBUF once; per 128-query chunk: scores in PSUM, exp(s - 50)
           with fused row-sum on ACT (softmax normalization deferred into the
           attn evacuation), probs -> probsT (PE transpose), attn,
           r1 = x + attn, LN1 *stats only*, r1T -> DRAM scratch.
  Phase C: w0/w1 resident.  LN1 is an affine per-token map, so
           LN1(r1) @ w0 = rstd1*(r1 @ w0) + (-m1*rstd1)*colsum(w0): the
           normalization folds into the e-psum evacuation (colsum via a
           ones-matmul, once).  Same for LN2: h -> hT unnormalized, stats
           only.  e kept in DRAM for the phase-D residual.
  Phase D: w2 resident; logits folded the same way; out = LN3(lrelu(. + e)).

(The LN-folding fast path requires the layernorm gains to be 1; otherwise a
general path normalizes in place before transposing.)

All matmuls run in float32r (HW-measured ~1.6e-4 matmul relative error, full
PE rate at free dim 512).
"""

import sys
from contextlib import ExitStack

import numpy as np

if "/opt/trn_rl_repo" not in sys.path:
    sys.path.insert(0, "/opt/trn_rl_repo")

import concourse.bass as bass
import concourse.mybir as mybir
import concourse.tile as tile
from concourse import bacc
from concourse.bass_utils import run_bass_kernel_spmd
from concourse.masks import make_identity

P = 128
S = 2048
D = 1024
H = 2048
N_CORES = 8
EPS = 1e-5
EXP_SHIFT = -50.0

FP32 = mybir.dt.float32
F32R = mybir.dt.float32r
AF = mybir.ActivationFunctionType
ALU = mybir.AluOpType

SD = S // P   # 16 token tiles
DD = D // P   # 8 feature tiles
HD = H // P   # 16 hidden tiles


def _mm(nc, out, lhsT, rhs, start, stop):
    nc.tensor.matmul(out, lhsT, rhs, start=start, stop=stop)


def _ln_stats(nc, pool, out2_ap, in_ap, n, eps_sb):
    """Write per-token rstd into out2_ap[:, 0:1] and -mean*rstd into
    out2_ap[:, 1:2] for a token-major [P, n] input."""
    nsub = n // 512
    stats = pool.tile([P, nsub, 6], FP32, tag="ln_stats")
    in3 = in_ap.rearrange("p (ns f) -> p ns f", ns=nsub)
    for i in range(nsub):
        nc.vector.bn_stats(stats[:, i, :], in3[:, i, :])
    mv = pool.tile([P, 2], FP32, tag="ln_mv")
    nc.vector.bn_aggr(mv, stats)
    rstd = out2_ap[:, 0:1]
    nc.scalar.activation(rstd, mv[:, 1:2], AF.Sqrt, bias=eps_sb, scale=1.0)
    nc.vector.reciprocal(rstd, rstd)
    nc.vector.tensor_scalar(out2_ap[:, 1:2], mv[:, 0:1], rstd, -1.0,
                            ALU.mult, ALU.mult)


def _layernorm(nc, pool, out_ap, in_ap, n, eps_sb, g_bcast=None, b_bcast=None):
    """Full token-major layernorm (stats + normalize)."""
    ln2 = pool.tile([P, 2], FP32, tag="ln_sc")
    _ln_stats(nc, pool, ln2, in_ap, n, eps_sb)
    nc.vector.tensor_scalar(out_ap, in_ap, ln2[:, 0:1], ln2[:, 1:2],
                            ALU.mult, ALU.add)
    if g_bcast is not None:
        nc.vector.tensor_mul(out_ap, out_ap, g_bcast)
    if b_bcast is not None:
        nc.vector.tensor_add(out_ap, out_ap, b_bcast)


def _lrelu(nc, out_ap, in_ap):
    # HW-verified exact leaky relu on the scalar engine
    nc.scalar.activation(out_ap, in_ap, AF.Lrelu, bias=0.0, scale=1.0, alpha=0.01)


def _bcast_load(nc, pool, dram_vec_ap, n, tag):
    """DMA-broadcast a [n] DRAM vector across all 128 partitions -> [P, n]."""
    t = pool.tile([P, n], FP32, tag=tag)
    src = bass.AP(
        tensor=dram_vec_ap.tensor,
        offset=dram_vec_ap.offset,
        ap=[[0, P]] + list(dram_vec_ap.ap),
    )
    nc.gpsimd.dma_start(out=t, in_=src)
    return t


def build_kernel(trivial):
    """trivial: dict name -> bool (bias all-zero / gain all-one at call time)."""
    # The LN2-folding fast path needs gain == 1 and bias == 0.
    fold2 = trivial["ln_g"] and trivial["ln_b"]

    nc = bacc.Bacc(None, target_bir_lowering=False)

    x_d = nc.dram_tensor("x", [S, D], FP32, kind="ExternalInput")
    wq_d = nc.dram_tensor("wq", [D, D], FP32, kind="ExternalInput")
    wk_d = nc.dram_tensor("wk", [D, D], FP32, kind="ExternalInput")
    wv_d = nc.dram_tensor("wv", [D, D], FP32, kind="ExternalInput")
    w0_d = nc.dram_tensor("w0", [D, D], FP32, kind="ExternalInput")
    w1_d = nc.dram_tensor("w1", [D, H], FP32, kind="ExternalInput")
    w2_d = nc.dram_tensor("w2", [H, D], FP32, kind="ExternalInput")
    vecs = {}
    for name, n in [
        ("bq", D), ("bk", D), ("bv", D), ("b0", D), ("b1", H), ("b2", D),
        ("n1_g", D), ("n1_b", D), ("ln_g", H), ("ln_b", H),
        ("n2_g", D), ("n2_b", D),
    ]:
        if not trivial[name]:
            vecs[name] = nc.dram_tensor(name, [n], FP32, kind="ExternalInput")
    out_d = nc.dram_tensor("out", [S, D], FP32, kind="ExternalOutput")

    with tile.TileContext(nc) as tc, ExitStack() as ctx:
        singles = ctx.enter_context(tc.tile_pool(name="singles", bufs=1))
        dram = ctx.enter_context(tc.tile_pool(name="dram", bufs=1, space="DRAM"))

        ident = singles.tile([P, P], FP32, tag="ident")
        make_identity(nc, ident)
        eps_sb = singles.tile([P, 1], FP32, tag="eps")
        nc.vector.memset(eps_sb, EPS)
        shift_sb = singles.tile([P, 1], FP32, tag="shift")
        nc.vector.memset(shift_sb, EXP_SHIFT)
        ones_f = singles.tile([P, P], FP32, tag="ones_f")
        nc.vector.memset(ones_f, 1.0)
        ones_r = singles.tile([P, P], F32R, tag="ones_r")
        nc.vector.tensor_copy(ones_r, ones_f)

        # Per-chunk DRAM scratch tiles (separate tiles let later phases
        # start on a chunk as soon as the producing phase finishes it).
        qT_ds = [dram.tile([DD, P, 512], F32R, tag=f"qT{i}", name=f"qT{i}")
                 for i in range(4)]
        r1T_ds = [dram.tile([DD, P, P], F32R, tag=f"r1T{i}", name=f"r1T{i}")
                  for i in range(SD)]
        e_ds = [dram.tile([P, D], FP32, tag=f"e{i}", name=f"e{i}")
                for i in range(SD)]
        hT_ds = [dram.tile([HD, P, P], F32R, tag=f"hT{i}", name=f"hT{i}")
                 for i in range(SD)]
        ln2_ds = [dram.tile([P, 2], FP32, tag=f"ln2_{i}", name=f"ln2_{i}")
                  for i in range(SD)]
        kT_d = dram.tile([DD, P, S], F32R, tag="kT_scr", name="kT_scr")

        x3 = x_d[:, :].rearrange("(st p) d -> st p d", p=P)

        # ============ Phases A+B: v resident in SBUF throughout ============
        with ExitStack() as ab:
            persist = ab.enter_context(tc.tile_pool(name="persistAB", bufs=1))
            v_sb = persist.tile([P, SD, D], F32R, tag="v")      # 64KB/part

            # ---------------- Phase A ----------------
            # Full xT resident so each weight slab streams exactly once.
            with ExitStack() as pa:
                pool = pa.enter_context(tc.tile_pool(name="phA", bufs=3))
                xTp = pa.enter_context(tc.tile_pool(name="phA_xT", bufs=1))
                wpool = pa.enter_context(tc.tile_pool(name="phA_w", bufs=2))
                pp_qk = pa.enter_context(
                    tc.tile_pool(name="ppA_qk", bufs=2, space="PSUM"))
                pp_v = pa.enter_context(
                    tc.tile_pool(name="ppA_v", bufs=2, space="PSUM"))
                pp_t = pa.enter_context(
                    tc.tile_pool(name="ppA_t", bufs=2, space="PSUM"))

                bq_pc = bk_pc = bv_bc = None
                if not trivial["bq"]:
                    bq_pc = pool.tile([P, DD], FP32, tag="bq_pc")
                    nc.sync.dma_start(
                        bq_pc, vecs["bq"][:].rearrange("(o p) -> p o", p=P))
                if not trivial["bk"]:
                    bk_pc = pool.tile([P, DD], FP32, tag="bk_pc")
                    nc.sync.dma_start(
                        bk_pc, vecs["bk"][:].rearrange("(o p) -> p o", p=P))
                if not trivial["bv"]:
                    bv_bc = _bcast_load(nc, pool, vecs["bv"][:], D, "bv_bc")

                # x -> xT (full [D, S] resident, 64KB/part)
                xT = xTp.tile([P, DD, S], F32R, tag="xT")
                for ss in range(SD):
                    xt = pool.tile([P, D], FP32, tag="x_in")
                    nc.sync.dma_start(xt, x3[ss])
                    for dk in range(DD):
                        ps = pp_t.tile([P, P], FP32, tag="tr")
                        nc.tensor.transpose(
                            ps, xt[:, dk * P:(dk + 1) * P], ident)
                        nc.vector.tensor_copy(
                            xT[:, dk, ss * P:(ss + 1) * P], ps)

                # kT first (phase B prefetches it), then qT, then v (v is
                # only needed once phase B reaches the attn matmuls)
                for w_d, kind, bias_pc in (
                        (wk_d, "k", bk_pc), (wq_d, "q", bq_pc),
                        (wv_d, "v", bv_bc)):
                    if kind in ("k", "q"):
                        # feature-major out: lhsT = weight slab slice
                        for half in range(2):
                            slab = wpool.tile([P, DD, 512], F32R, tag="wslab")
                            nc.gpsimd.dma_start(
                                out=slab,
                                in_=w_d[:, half * 512:(half + 1) * 512]
                                .rearrange("(ko p) n -> p ko n", p=P))
                            for dml in range(4):
                                dm = half * 4 + dml
                                for sc in range(4):
                                    ps = pp_qk.tile([P, 512], FP32, tag="qk")
                                    for k in range(DD):
                                        _mm(nc, ps,
                                            slab[:, k, dml * P:(dml + 1) * P],
                                            xT[:, k, sc * 512:(sc + 1) * 512],
                                            start=(k == 0), stop=(k == DD - 1))
                                    st_t = pool.tile([P, 512], F32R,
                                                     tag="kq_st")
                                    if bias_pc is None:
                                        nc.scalar.copy(st_t, ps)
                                    else:
                                        nc.scalar.activation(
                                            st_t, ps, AF.Identity,
                                            bias=bias_pc[:, dm:dm + 1],
                                            scale=1.0)
                                    if kind == "k":
                                        nc.sync.dma_start(
                                            kT_d[dm, :, sc * 512:(sc + 1) * 512],
                                            st_t)
                                    else:
                                        nc.sync.dma_start(
                                            qT_ds[sc][dm, :, :], st_t)
                    else:
                        # v (token-major): lhsT = xT subtile, rhs = wv slab
                        for dn in range(D // 512):
                            slab = wpool.tile([P, DD, 512], F32R, tag="wslab")
                            nc.gpsimd.dma_start(
                                out=slab,
                                in_=w_d[:, dn * 512:(dn + 1) * 512]
                                .rearrange("(ko p) n -> p ko n", p=P))
                            for ss in range(SD):
                                ps = pp_v.tile([P, 512], FP32, tag="vps")
                                for k in range(DD):
                                    _mm(nc, ps,
                                        xT[:, k, ss * P:(ss + 1) * P],
                                        slab[:, k, :],
                                        start=(k == 0), stop=(k == DD - 1))
                                dst = v_sb[:, ss, dn * 512:(dn + 1) * 512]
                                if bv_bc is not None:
                                    nc.vector.tensor_add(
                                        dst, ps,
                                        bv_bc[:, dn * 512:(dn + 1) * 512])
                                else:
                                    nc.vector.tensor_copy(dst, ps)

            # ---------------- Phase B ----------------
            with ExitStack() as pb:
                kTp = pb.enter_context(tc.tile_pool(name="phB_kT", bufs=1))
                kT_sb = kTp.tile([P, DD, S], F32R, tag="kT")    # 64KB/part
                nc.sync.dma_start(
                    kT_sb, kT_d[:, :, :].rearrange("dk p s -> p dk s"))

                pool = pb.enter_context(tc.tile_pool(name="phB", bufs=2))
                pool1 = pb.enter_context(tc.tile_pool(name="phB1", bufs=1))
                small = pb.enter_context(tc.tile_pool(name="phB_small", bufs=4))
                pp_s = pb.enter_context(
                    tc.tile_pool(name="ppB_s", bufs=1, space="PSUM"))
                pp_a = pb.enter_context(
                    tc.tile_pool(name="ppB_a", bufs=1, space="PSUM"))
                pp_t = pb.enter_context(
                    tc.tile_pool(name="ppB_t", bufs=2, space="PSUM"))

                n1g_bc = n1b_bc = None
                if not trivial["n1_g"]:
                    n1g_bc = _bcast_load(nc, pool1, vecs["n1_g"][:], D, "n1g_bc")
                if not trivial["n1_b"]:
                    n1b_bc = _bcast_load(nc, pool1, vecs["n1_b"][:], D, "n1b_bc")

                TN = S // 512  # 4 score column blocks
                for st in range(SD):  # 16 chunks of 128 queries
                    qT = pool.tile([P, DD, P], F32R, tag="qT")
                    nc.sync.dma_start(
                        qT,
                        qT_ds[st // 4][:, :, (st % 4) * P:(st % 4 + 1) * P]
                        .rearrange("dk p s -> p dk s"))

                    probs = pool1.tile([P, S], FP32, tag="probs")
                    den4 = small.tile([P, TN], FP32, tag="den4")
                    for tn in range(TN):
                        ps_s = pp_s.tile([P, 512], FP32, tag=f"sc{tn}",
                                         name=f"pssc{tn}")
                        for k in range(DD):
                            _mm(nc, ps_s, qT[:, k, :],
                                kT_sb[:, k, tn * 512:(tn + 1) * 512],
                                start=(k == 0), stop=(k == DD - 1))
                        # exp(s - 50) with fused row-sum; normalization is
                        # folded into the attn evacuation below
                        nc.scalar.activation(
                            probs[:, tn * 512:(tn + 1) * 512], ps_s,
                            AF.Exp, bias=shift_sb, scale=1.0,
                            accum_out=den4[:, tn:tn + 1])
                    denom = small.tile([P, 1], FP32, tag="denom")
                    nc.vector.reduce_sum(denom, den4, axis=mybir.AxisListType.X)
                    rden = small.tile([P, 1], FP32, tag="rden")
                    nc.vector.reciprocal(rden, denom)

                    # probsT via PE transpose: [P t, SD, P s]
                    probsT = pool1.tile([P, SD, P], F32R, tag="probsT")
                    for tt in range(SD):
                        ps = pp_t.tile([P, P], FP32, tag="tr")
                        nc.tensor.transpose(
                            ps, probs[:, tt * P:(tt + 1) * P], ident)
                        nc.vector.tensor_copy(probsT[:, tt, :], ps)

                    # attn = (probs @ v) * rden ; r1 = x + attn (in place)
                    r1 = pool.tile([P, D], FP32, tag="r1")
                    nc.sync.dma_start(r1, x3[st])
                    psa = [pp_a.tile([P, 512], FP32, tag=f"at{dn}",
                                     name=f"psat{dn}")
                           for dn in range(2)]
                    for tt in range(SD):
                        for dn in range(2):
                            _mm(nc, psa[dn], probsT[:, tt, :],
                                v_sb[:, tt, dn * 512:(dn + 1) * 512],
                                start=(tt == 0), stop=(tt == SD - 1))
                    for dn in range(2):
                        nc.vector.scalar_tensor_tensor(
                            r1[:, dn * 512:(dn + 1) * 512], psa[dn], rden,
                            r1[:, dn * 512:(dn + 1) * 512],
                            op0=ALU.mult, op1=ALU.add)

                    # LN1 (full): h1 = normalize(r1) * g + b
                    h1 = pool.tile([P, D], FP32, tag="h1")
                    _layernorm(nc, small, h1, r1, D, eps_sb, n1g_bc, n1b_bc)

                    # h1 -> h1T -> DRAM scratch
                    r1T = pool.tile([P, DD, P], F32R, tag="r1T")
                    for dk in range(DD):
                        ps = pp_t.tile([P, P], FP32, tag="tr")
                        nc.tensor.transpose(
                            ps, h1[:, dk * P:(dk + 1) * P], ident)
                        nc.scalar.copy(r1T[:, dk, :], ps)
                    nc.sync.dma_start(
                        r1T_ds[st][:, :, :].rearrange("dk p s -> p dk s"), r1T)

        # ---------------- Phase C: w0, w1 resident ----------------
        with ExitStack() as pc:
            wres = pc.enter_context(tc.tile_pool(name="phC_w", bufs=1))
            pool = pc.enter_context(tc.tile_pool(name="phC", bufs=2))
            pool3 = pc.enter_context(tc.tile_pool(name="phC3", bufs=3))
            small = pc.enter_context(tc.tile_pool(name="phC_small", bufs=4))
            pp_e = pc.enter_context(
                tc.tile_pool(name="ppC_e", bufs=1, space="PSUM"))
            pp_h = pc.enter_context(
                tc.tile_pool(name="ppC_h", bufs=1, space="PSUM"))
            pp_t = pc.enter_context(
                tc.tile_pool(name="ppC_t", bufs=2, space="PSUM"))

            w0_sb = wres.tile([P, DD, D], F32R, tag="w0")   # 32KB/part
            nc.gpsimd.dma_start(
                out=w0_sb, in_=w0_d[:, :].rearrange("(ko p) n -> p ko n", p=P))
            w1_sb = wres.tile([P, DD, H], F32R, tag="w1")   # 64KB/part
            nc.gpsimd.dma_start(
                out=w1_sb, in_=w1_d[:, :].rearrange("(ko p) n -> p ko n", p=P))

            b0_bc = b1_bc = lng_bc = lnb_bc = None
            if not trivial["b0"]:
                b0_bc = _bcast_load(nc, pool, vecs["b0"][:], D, "b0_bc")
            if not trivial["b1"]:
                b1_bc = _bcast_load(nc, pool, vecs["b1"][:], H, "b1_bc")
            if not trivial["ln_g"]:
                lng_bc = _bcast_load(nc, pool, vecs["ln_g"][:], H, "lng_bc")
            if not trivial["ln_b"]:
                lnb_bc = _bcast_load(nc, pool, vecs["ln_b"][:], H, "lnb_bc")

            for st in range(SD):
                r1T = pool3.tile([P, DD, P], F32R, tag="r1T")
                nc.sync.dma_start(
                    r1T, r1T_ds[st][:, :, :].rearrange("dk p s -> p dk s"))

                # e = h1 @ w0 + b0 (h1 already LN1-normalized)
                e_sb = pool3.tile([P, D], FP32, tag="e")
                pse = [pp_e.tile([P, 512], FP32, tag=f"e{dn}", name=f"pse{dn}")
                       for dn in range(2)]
                for k in range(DD):
                    for dn in range(2):
                        _mm(nc, pse[dn], r1T[:, k, :],
                            w0_sb[:, k, dn * 512:(dn + 1) * 512],
                            start=(k == 0), stop=(k == DD - 1))
                for dn in range(2):
                    dst = e_sb[:, dn * 512:(dn + 1) * 512]
                    nc.scalar.copy(dst, pse[dn])
                    if b0_bc is not None:
                        nc.vector.tensor_add(
                            dst, dst, b0_bc[:, dn * 512:(dn + 1) * 512])
                nc.sync.dma_start(e_ds[st][:, :], e_sb)

                # eT via PE transpose (evacuate on DVE to offload ACT)
                eT = pool3.tile([P, DD, P], F32R, tag="eT")
                for dk in range(DD):
                    ps = pp_t.tile([P, P], FP32, tag="tr")
                    nc.tensor.transpose(ps, e_sb[:, dk * P:(dk + 1) * P], ident)
                    nc.vector.tensor_copy(eT[:, dk, :], ps)

                # h = lrelu(e @ w1 + b1); LN2 stats only (fold path)
                h_sb = pool.tile([P, H], FP32, tag="h")
                psh = [pp_h.tile([P, 512], FP32, tag=f"h{hn}", name=f"psh{hn}")
                       for hn in range(4)]
                for k in range(DD):
                    for hn in range(4):
                        _mm(nc, psh[hn], eT[:, k, :],
                            w1_sb[:, k, hn * 512:(hn + 1) * 512],
                            start=(k == 0), stop=(k == DD - 1))
                for hn in range(4):
                    dst = h_sb[:, hn * 512:(hn + 1) * 512]
                    if b1_bc is not None:
                        nc.vector.tensor_add(
                            dst, psh[hn], b1_bc[:, hn * 512:(hn + 1) * 512])
                        _lrelu(nc, dst, dst)
                    else:
                        _lrelu(nc, dst, psh[hn])

                ln2 = small.tile([P, 2], FP32, tag="ln2")
                _ln_stats(nc, small, ln2, h_sb, H, eps_sb)
                nc.sync.dma_start(ln2_ds[st][:, :], ln2)
                if fold2:
                    tr2_src = h_sb
                else:
                    h2 = pool.tile([P, H], FP32, tag="h2")
                    nc.vector.tensor_scalar(h2, h_sb, ln2[:, 0:1],
                                            ln2[:, 1:2], ALU.mult, ALU.add)
                    if lng_bc is not None:
                        nc.vector.tensor_mul(h2, h2, lng_bc)
                    if lnb_bc is not None:
                        nc.vector.tensor_add(h2, h2, lnb_bc)
                    tr2_src = h2

                # h -> hT -> DRAM scratch (unnormalized on fold path)
                hT = pool.tile([P, HD, P], F32R, tag="hT")
                for hk in range(HD):
                    ps = pp_t.tile([P, P], FP32, tag="tr")
                    nc.tensor.transpose(
                        ps, tr2_src[:, hk * P:(hk + 1) * P], ident)
                    if hk % 2 == 0:
                        nc.vector.tensor_copy(hT[:, hk, :], ps)
                    else:
                        nc.scalar.copy(hT[:, hk, :], ps)
                nc.sync.dma_start(
                    hT_ds[st][:, :, :].rearrange("hk p s -> p hk s"), hT)

        # ---------------- Phase D: w2 resident ----------------
        with ExitStack() as pd:
            wres = pd.enter_context(tc.tile_pool(name="phD_w", bufs=1))
            pool = pd.enter_context(tc.tile_pool(name="phD", bufs=3))
            small = pd.enter_context(tc.tile_pool(name="phD_small", bufs=4))
            pp = pd.enter_context(tc.tile_pool(name="ppD", bufs=1, space="PSUM"))

            w2_sb = wres.tile([P, HD, D], F32R, tag="w2")   # 64KB/part
            nc.gpsimd.dma_start(
                out=w2_sb, in_=w2_d[:, :].rearrange("(ko p) n -> p ko n", p=P))

            b2_bc = n2g_bc = n2b_bc = None
            if not trivial["b2"]:
                b2_bc = _bcast_load(nc, pool, vecs["b2"][:], D, "b2_bc")
            if not trivial["n2_g"]:
                n2g_bc = _bcast_load(nc, pool, vecs["n2_g"][:], D, "n2g_bc")
            if not trivial["n2_b"]:
                n2b_bc = _bcast_load(nc, pool, vecs["n2_b"][:], D, "n2b_bc")

            # colsum(w2) broadcast over partitions (fold path)
            w2s_bc = None
            if fold2:
                w2s_bc = wres.tile([P, D], FP32, tag="w2s")
                for dn in range(2):
                    ps = pp.tile([P, 512], FP32, tag="l0", name="ps_w2s")
                    for k in range(HD):
                        _mm(nc, ps, ones_r, w2_sb[:, k, dn * 512:(dn + 1) * 512],
                            start=(k == 0), stop=(k == HD - 1))
                    nc.vector.tensor_copy(w2s_bc[:, dn * 512:(dn + 1) * 512], ps)

            for st in range(SD):
                hT = pool.tile([P, HD, P], F32R, tag="hT")
                nc.sync.dma_start(
                    hT, hT_ds[st][:, :, :].rearrange("hk p s -> p hk s"))
                e_sb = pool.tile([P, D], FP32, tag="e")
                nc.sync.dma_start(e_sb, e_ds[st][:, :])
                ln2 = small.tile([P, 2], FP32, tag="ln2")
                nc.sync.dma_start(ln2, ln2_ds[st][:, :])

                t_sb = pool.tile([P, D], FP32, tag="t")
                psl = [pp.tile([P, 512], FP32, tag=f"l{dn}", name=f"psl{dn}")
                       for dn in range(2)]
                for k in range(HD):
                    for dn in range(2):
                        _mm(nc, psl[dn], hT[:, k, :],
                            w2_sb[:, k, dn * 512:(dn + 1) * 512],
                            start=(k == 0), stop=(k == HD - 1))
                if fold2:
                    # t = rstd2*(h @ w2) + nmr2*colsum(w2) + b2 + e
                    ltmp = pool.tile([P, D], FP32, tag="ltmp")
                    nc.vector.tensor_scalar(ltmp, w2s_bc, ln2[:, 1:2], None,
                                            ALU.mult)
                    nc.vector.tensor_add(ltmp, ltmp, e_sb)
                    if b2_bc is not None:
                        nc.vector.tensor_add(ltmp, ltmp, b2_bc)
                    for dn in range(2):
                        nc.vector.scalar_tensor_tensor(
                            t_sb[:, dn * 512:(dn + 1) * 512], psl[dn],
                            ln2[:, 0:1], ltmp[:, dn * 512:(dn + 1) * 512],
                            op0=ALU.mult, op1=ALU.add)
                else:
                    for dn in range(2):
                        dst = t_sb[:, dn * 512:(dn + 1) * 512]
                        nc.vector.tensor_add(
                            dst, psl[dn], e_sb[:, dn * 512:(dn + 1) * 512])
                        if b2_bc is not None:
                            nc.vector.tensor_add(
                                dst, dst, b2_bc[:, dn * 512:(dn + 1) * 512])
                _lrelu(nc, t_sb, t_sb)

                o_sb = pool.tile([P, D], FP32, tag="o")
                _layernorm(nc, small, o_sb, t_sb, D, eps_sb, n2g_bc, n2b_bc)
                nc.sync.dma_start(out_d[st * P:(st + 1) * P, :], o_sb)

    nc.compile()
    return nc


_CACHE = {}


def kernel(**inputs):
    x_emb = np.ascontiguousarray(inputs["x_embeddings"], dtype=np.float32)
    B = x_emb.shape[0]
    assert x_emb.shape == (B, S, D)

    trivial = {}
    for name in ["bq", "bk", "bv", "b0", "b1", "b2", "n1_b", "ln_b", "n2_b"]:
        trivial[name] = bool(np.all(np.asarray(inputs[name]) == 0.0))
    for name in ["n1_g", "ln_g", "n2_g"]:
        trivial[name] = bool(np.all(np.asarray(inputs[name]) == 1.0))

    key = tuple(sorted(trivial.items()))
    if key not in _CACHE:
        _CACHE[key] = build_kernel(trivial)
    nc = _CACHE[key]

    shared = {
        name: np.ascontiguousarray(inputs[name], dtype=np.float32)
        for name in ["wq", "wk", "wv", "w0", "w1", "w2"]
    }
    for name, triv in trivial.items():
        if not triv:
            shared[name] = np.ascontiguousarray(inputs[name], dtype=np.float32)

    in_maps = [dict(shared, x=x_emb[b]) for b in range(B)]
    res = run_bass_kernel_spmd(nc, in_maps, core_ids=list(range(N_CORES)))
    out = np.stack([res.results[b]["out"] for b in range(B)], axis=0)
    return out.astype(np.float32)


# revision 17
# speedup vs baseline: 1.1334x; 1.1334x over previous
"""Trainium2 Bass kernel for nn_AttentionLayer (dense transformer layer).

Reference computation (per batch b):
    q = x @ wq + bq ; k = x @ wk + bk ; v = x @ wv + bv
    scores = q @ k.T              (no scaling, no mask)
    probs  = softmax(scores, -1)
    attn   = probs @ v
    e      = LN1(x + attn) @ w0 + b0
    h      = LN2(lrelu(e @ w1 + b1))
    logits = h @ w2 + b2
    out    = LN3(lrelu(logits + e))

Sharding: data-parallel over batch. B=8 batches -> 8 NeuronCores, one batch
per core, weights replicated.  No collectives.

Per-core schedule (S=2048, D=1024, H=2048, P=128):
  Phase A: x -> xT (PE transpose, full [D,S] resident); weights streamed once
           as fp32r slabs via gpsimd casting DMA; kT -> DRAM scratch,
           qT -> DRAM scratch, v -> resident SBUF.
  Phase B: kT -> SBUF once; per 128-query chunk: scores in PSUM, exp(s - 50)
           with fused row-sum on ACT (softmax normalization deferred into the
           attn evacuation), probs -> probsT (PE transpose), attn,
           r1 = x + attn, LN1 *stats only*, r1T -> DRAM scratch.
  Phase C: w0/w1 resident.  LN1 is an affine per-token map, so
           LN1(r1) @ w0 = rstd1*(r1 @ w0) + (-m1*rstd1)*colsum(w0): the
           normalization folds into the e-psum evacuation (colsum via a
           ones-matmul, once).  Same for LN2: h -> hT unnormalized, stats
           only.  e kept in DRAM for the phase-D residual.
  Phase D: w2 resident; logits folded the same way; out = LN3(lrelu(. + e)).

(The LN-folding fast path requires the layernorm gains to be 1; otherwise a
general path normalizes in place before transposing.)

All matmuls run in float32r (HW-measured ~1.6e-4 matmul relative error, full
PE rate at free dim 512).
"""

import sys
from contextlib import ExitStack

import numpy as np

if "/opt/trn_rl_repo" not in sys.path:
    sys.path.insert(0, "/opt/trn_rl_repo")

import concourse.bass as bass
import concourse.mybir as mybir
import concourse.tile as tile
from concourse import bacc
from concourse.bass_utils import run_bass_kernel_spmd
from concourse.masks import make_identity

P = 128
S = 2048
D = 1024
H = 2048
N_CORES = 8
EPS = 1e-5
EXP_SHIFT = -50.0

FP32 = mybir.dt.float32
F32R = mybir.dt.float32r
AF = mybir.ActivationFunctionType
ALU = mybir.AluOpType

SD = S // P   # 16 token tiles
DD = D // P   # 8 feature tiles
HD = H // P   # 16 hidden tiles


def _mm(nc, out, lhsT, rhs, start, stop):
    nc.tensor.matmul(out, lhsT, rhs, start=start, stop=stop)


def _ln_stats(nc, pool, out2_ap, in_ap, n, eps_sb):
    """Write per-token rstd into out2_ap[:, 0:1] and -mean*rstd into
    out2_ap[:, 1:2] for a token-major [P, n] input."""
    nsub = n // 512
    stats = pool.tile([P, nsub, 6], FP32, tag="ln_stats")
    in3 = in_ap.rearrange("p (ns f) -> p ns f", ns=nsub)
    for i in range(nsub):
        nc.vector.bn_stats(stats[:, i, :], in3[:, i, :])
    mv = pool.tile([P, 2], FP32, tag="ln_mv")
    nc.vector.bn_aggr(mv, stats)
    rstd = out2_ap[:, 0:1]
    nc.scalar.activation(rstd, mv[:, 1:2], AF.Sqrt, bias=eps_sb, scale=1.0)
    nc.vector.reciprocal(rstd, rstd)
    nc.vector.tensor_scalar(out2_ap[:, 1:2], mv[:, 0:1], rstd, -1.0,
                            ALU.mult, ALU.mult)


def _layernorm(nc, pool, out_ap, in_ap, n, eps_sb, g_bcast=None, b_bcast=None):
    """Full token-major layernorm (stats + normalize)."""
    ln2 = pool.tile([P, 2], FP32, tag="ln_sc")
    _ln_stats(nc, pool, ln2, in_ap, n, eps_sb)
    nc.vector.tensor_scalar(out_ap, in_ap, ln2[:, 0:1], ln2[:, 1:2],
                            ALU.mult, ALU.add)
    if g_bcast is not None:
        nc.vector.tensor_mul(out_ap, out_ap, g_bcast)
    if b_bcast is not None:
        nc.vector.tensor_add(out_ap, out_ap, b_bcast)


def _lrelu(nc, out_ap, in_ap):
    # HW-verified exact leaky relu on the scalar engine
    nc.scalar.activation(out_ap, in_ap, AF.Lrelu, bias=0.0, scale=1.0, alpha=0.01)


def _bcast_load(nc, pool, dram_vec_ap, n, tag):
    """DMA-broadcast a [n] DRAM vector across all 128 partitions -> [P, n]."""
    t = pool.tile([P, n], FP32, tag=tag)
    src = bass.AP(
        tensor=dram_vec_ap.tensor,
        offset=dram_vec_ap.offset,
        ap=[[0, P]] + list(dram_vec_ap.ap),
    )
    nc.gpsimd.dma_start(out=t, in_=src)
    return t


def build_kernel(trivial):
    """trivial: dict name -> bool (bias all-zero / gain all-one at call time)."""
    # The LN2-folding fast path needs gain == 1 and bias == 0.
    fold2 = trivial["ln_g"] and trivial["ln_b"]

    nc = bacc.Bacc(None, target_bir_lowering=False)

    x_d = nc.dram_tensor("x", [S, D], FP32, kind="ExternalInput")
    wq_d = nc.dram_tensor("wq", [D, D], FP32, kind="ExternalInput")
    wk_d = nc.dram_tensor("wk", [D, D], FP32, kind="ExternalInput")
    wv_d = nc.dram_tensor("wv", [D, D], FP32, kind="ExternalInput")
    w0_d = nc.dram_tensor("w0", [D, D], FP32, kind="ExternalInput")
    w1_d = nc.dram_tensor("w1", [D, H], FP32, kind="ExternalInput")
    w2_d = nc.dram_tensor("w2", [H, D], FP32, kind="ExternalInput")
    vecs = {}
    for name, n in [
        ("bq", D), ("bk", D), ("bv", D), ("b0", D), ("b1", H), ("b2", D),
        ("n1_g", D), ("n1_b", D), ("ln_g", H), ("ln_b", H),
        ("n2_g", D), ("n2_b", D),
    ]:
        if not trivial[name]:
            vecs[name] = nc.dram_tensor(name, [n], FP32, kind="ExternalInput")
    out_d = nc.dram_tensor("out", [S, D], FP32, kind="ExternalOutput")

    with tile.TileContext(nc) as tc, ExitStack() as ctx:
        singles = ctx.enter_context(tc.tile_pool(name="singles", bufs=1))
        dram = ctx.enter_context(tc.tile_pool(name="dram", bufs=1, space="DRAM"))

        ident = singles.tile([P, P], FP32, tag="ident")
        make_identity(nc, ident)
        eps_sb = singles.tile([P, 1], FP32, tag="eps")
        nc.vector.memset(eps_sb, EPS)
        shift_sb = singles.tile([P, 1], FP32, tag="shift")
        nc.vector.memset(shift_sb, EXP_SHIFT)
        ones_f = singles.tile([P, P], FP32, tag="ones_f")
        nc.vector.memset(ones_f, 1.0)
        ones_r = singles.tile([P, P], F32R, tag="ones_r")
        nc.vector.tensor_copy(ones_r, ones_f)

        # Per-chunk DRAM scratch tiles (separate tiles let later phases
        # start on a chunk as soon as the producing phase finishes it).
        qT_ds = [dram.tile([DD, P, 512], F32R, tag=f"qT{i}", name=f"qT{i}")
                 for i in range(4)]
        r1T_ds = [dram.tile([DD, P, P], F32R, tag=f"r1T{i}", name=f"r1T{i}")
                  for i in range(SD)]
        e_ds = [dram.tile([P, D], FP32, tag=f"e{i}", name=f"e{i}")
                for i in range(SD)]
        eT_ds = [dram.tile([DD, P, P], F32R, tag=f"eT{i}", name=f"eT{i}")
                 for i in range(SD)]
        kT_d = dram.tile([DD, P, S], F32R, tag="kT_scr", name="kT_scr")

        x3 = x_d[:, :].rearrange("(st p) d -> st p d", p=P)

        # ============ Phases A+B: v resident in SBUF throughout ============
        with ExitStack() as ab:
            persist = ab.enter_context(tc.tile_pool(name="persistAB", bufs=1))
            v_sb = persist.tile([P, SD, D], F32R, tag="v")      # 64KB/part

            # ---------------- Phase A ----------------
            # Full xT resident so each weight slab streams exactly once.
            with ExitStack() as pa:
                pool = pa.enter_context(tc.tile_pool(name="phA", bufs=3))
                xTp = pa.enter_context(tc.tile_pool(name="phA_xT", bufs=1))
                wpool = pa.enter_context(tc.tile_pool(name="phA_w", bufs=2))
                pp_qk = pa.enter_context(
                    tc.tile_pool(name="ppA_qk", bufs=2, space="PSUM"))
                pp_v = pa.enter_context(
                    tc.tile_pool(name="ppA_v", bufs=2, space="PSUM"))
                pp_t = pa.enter_context(
                    tc.tile_pool(name="ppA_t", bufs=2, space="PSUM"))

                bq_pc = bk_pc = bv_bc = None
                if not trivial["bq"]:
                    bq_pc = pool.tile([P, DD], FP32, tag="bq_pc")
                    nc.sync.dma_start(
                        bq_pc, vecs["bq"][:].rearrange("(o p) -> p o", p=P))
                if not trivial["bk"]:
                    bk_pc = pool.tile([P, DD], FP32, tag="bk_pc")
                    nc.sync.dma_start(
                        bk_pc, vecs["bk"][:].rearrange("(o p) -> p o", p=P))
                if not trivial["bv"]:
                    bv_bc = _bcast_load(nc, pool, vecs["bv"][:], D, "bv_bc")

                # x -> xT (full [D, S] resident, 64KB/part)
                xT = xTp.tile([P, DD, S], F32R, tag="xT")
                for ss in range(SD):
                    xt = pool.tile([P, D], FP32, tag="x_in")
                    nc.sync.dma_start(xt, x3[ss])
                    for dk in range(DD):
                        ps = pp_t.tile([P, P], FP32, tag="tr")
                        nc.tensor.transpose(
                            ps, xt[:, dk * P:(dk + 1) * P], ident)
                        nc.vector.tensor_copy(
                            xT[:, dk, ss * P:(ss + 1) * P], ps)

                # kT first (phase B prefetches it), then qT, then v (v is
                # only needed once phase B reaches the attn matmuls)
                for w_d, kind, bias_pc in (
                        (wk_d, "k", bk_pc), (wq_d, "q", bq_pc),
                        (wv_d, "v", bv_bc)):
                    if kind in ("k", "q"):
                        # feature-major out: lhsT = weight slab slice
                        for half in range(2):
                            slab = wpool.tile([P, DD, 512], F32R, tag="wslab")
                            nc.gpsimd.dma_start(
                                out=slab,
                                in_=w_d[:, half * 512:(half + 1) * 512]
                                .rearrange("(ko p) n -> p ko n", p=P))
                            for dml in range(4):
                                dm = half * 4 + dml
                                for sc in range(4):
                                    ps = pp_qk.tile([P, 512], FP32, tag="qk")
                                    for k in range(DD):
                                        _mm(nc, ps,
                                            slab[:, k, dml * P:(dml + 1) * P],
                                            xT[:, k, sc * 512:(sc + 1) * 512],
                                            start=(k == 0), stop=(k == DD - 1))
                                    st_t = pool.tile([P, 512], F32R,
                                                     tag="kq_st")
                                    if bias_pc is None:
                                        nc.scalar.copy(st_t, ps)
                                    else:
                                        nc.scalar.activation(
                                            st_t, ps, AF.Identity,
                                            bias=bias_pc[:, dm:dm + 1],
                                            scale=1.0)
                                    if kind == "k":
                                        nc.sync.dma_start(
                                            kT_d[dm, :, sc * 512:(sc + 1) * 512],
                                            st_t)
                                    else:
                                        nc.sync.dma_start(
                                            qT_ds[sc][dm, :, :], st_t)
                    else:
                        # v (token-major): lhsT = xT subtile, rhs = wv slab
                        for dn in range(D // 512):
                            slab = wpool.tile([P, DD, 512], F32R, tag="wslab")
                            nc.gpsimd.dma_start(
                                out=slab,
                                in_=w_d[:, dn * 512:(dn + 1) * 512]
                                .rearrange("(ko p) n -> p ko n", p=P))
                            for ss in range(SD):
                                ps = pp_v.tile([P, 512], FP32, tag="vps")
                                for k in range(DD):
                                    _mm(nc, ps,
                                        xT[:, k, ss * P:(ss + 1) * P],
                                        slab[:, k, :],
                                        start=(k == 0), stop=(k == DD - 1))
                                dst = v_sb[:, ss, dn * 512:(dn + 1) * 512]
                                if bv_bc is not None:
                                    nc.vector.tensor_add(
                                        dst, ps,
                                        bv_bc[:, dn * 512:(dn + 1) * 512])
                                else:
                                    nc.vector.tensor_copy(dst, ps)

            # ---------------- Phase B ----------------
            with ExitStack() as pb:
                kTp = pb.enter_context(tc.tile_pool(name="phB_kT", bufs=1))
                kT_sb = kTp.tile([P, DD, S], F32R, tag="kT")    # 64KB/part
                nc.sync.dma_start(
                    kT_sb, kT_d[:, :, :].rearrange("dk p s -> p dk s"))

                pool = pb.enter_context(tc.tile_pool(name="phB", bufs=2))
                pool1 = pb.enter_context(tc.tile_pool(name="phB1", bufs=1))
                small = pb.enter_context(tc.tile_pool(name="phB_small", bufs=4))
                pp_s = pb.enter_context(
                    tc.tile_pool(name="ppB_s", bufs=1, space="PSUM"))
                pp_a = pb.enter_context(
                    tc.tile_pool(name="ppB_a", bufs=1, space="PSUM"))
                pp_t = pb.enter_context(
                    tc.tile_pool(name="ppB_t", bufs=2, space="PSUM"))

                n1g_bc = n1b_bc = None
                if not trivial["n1_g"]:
                    n1g_bc = _bcast_load(nc, pool1, vecs["n1_g"][:], D, "n1g_bc")
                if not trivial["n1_b"]:
                    n1b_bc = _bcast_load(nc, pool1, vecs["n1_b"][:], D, "n1b_bc")

                TN = S // 512  # 4 score column blocks
                for st in range(SD):  # 16 chunks of 128 queries
                    qT = pool.tile([P, DD, P], F32R, tag="qT")
                    nc.sync.dma_start(
                        qT,
                        qT_ds[st // 4][:, :, (st % 4) * P:(st % 4 + 1) * P]
                        .rearrange("dk p s -> p dk s"))

                    probs = pool1.tile([P, S], FP32, tag="probs")
                    den4 = small.tile([P, TN], FP32, tag="den4")
                    for tn in range(TN):
                        ps_s = pp_s.tile([P, 512], FP32, tag=f"sc{tn}",
                                         name=f"pssc{tn}")
                        for k in range(DD):
                            _mm(nc, ps_s, qT[:, k, :],
                                kT_sb[:, k, tn * 512:(tn + 1) * 512],
                                start=(k == 0), stop=(k == DD - 1))
                        # exp(s - 50) with fused row-sum; normalization is
                        # folded into the attn evacuation below
                        nc.scalar.activation(
                            probs[:, tn * 512:(tn + 1) * 512], ps_s,
                            AF.Exp, bias=shift_sb, scale=1.0,
                            accum_out=den4[:, tn:tn + 1])
                    denom = small.tile([P, 1], FP32, tag="denom")
                    nc.vector.reduce_sum(denom, den4, axis=mybir.AxisListType.X)
                    rden = small.tile([P, 1], FP32, tag="rden")
                    nc.vector.reciprocal(rden, denom)

                    # probsT via PE transpose: [P t, SD, P s]
                    probsT = pool1.tile([P, SD, P], F32R, tag="probsT")
                    for tt in range(SD):
                        ps = pp_t.tile([P, P], FP32, tag="tr")
                        nc.tensor.transpose(
                            ps, probs[:, tt * P:(tt + 1) * P], ident)
                        nc.vector.tensor_copy(probsT[:, tt, :], ps)

                    # attn = (probs @ v) * rden ; r1 = x + attn (in place)
                    r1 = pool.tile([P, D], FP32, tag="r1")
                    nc.sync.dma_start(r1, x3[st])
                    psa = [pp_a.tile([P, 512], FP32, tag=f"at{dn}",
                                     name=f"psat{dn}")
                           for dn in range(2)]
                    for tt in range(SD):
                        for dn in range(2):
                            _mm(nc, psa[dn], probsT[:, tt, :],
                                v_sb[:, tt, dn * 512:(dn + 1) * 512],
                                start=(tt == 0), stop=(tt == SD - 1))
                    for dn in range(2):
                        nc.vector.scalar_tensor_tensor(
                            r1[:, dn * 512:(dn + 1) * 512], psa[dn], rden,
                            r1[:, dn * 512:(dn + 1) * 512],
                            op0=ALU.mult, op1=ALU.add)

                    # LN1 (full): h1 = normalize(r1) * g + b
                    h1 = pool.tile([P, D], FP32, tag="h1")
                    _layernorm(nc, small, h1, r1, D, eps_sb, n1g_bc, n1b_bc)

                    # h1 -> h1T -> DRAM scratch
                    r1T = pool.tile([P, DD, P], F32R, tag="r1T")
                    for dk in range(DD):
                        ps = pp_t.tile([P, P], FP32, tag="tr")
                        nc.tensor.transpose(
                            ps, h1[:, dk * P:(dk + 1) * P], ident)
                        nc.scalar.copy(r1T[:, dk, :], ps)
                    nc.sync.dma_start(
                        r1T_ds[st][:, :, :].rearrange("dk p s -> p dk s"), r1T)

        # ------------- Phase C1: e = h1 @ w0 (w0 resident) -------------
        with ExitStack() as pc1:
            wres = pc1.enter_context(tc.tile_pool(name="phC1_w", bufs=1))
            pool = pc1.enter_context(tc.tile_pool(name="phC1", bufs=4))
            pp_e = pc1.enter_context(
                tc.tile_pool(name="ppC1_e", bufs=2, space="PSUM"))
            pp_t = pc1.enter_context(
                tc.tile_pool(name="ppC1_t", bufs=2, space="PSUM"))

            w0_sb = wres.tile([P, DD, D], F32R, tag="w0")   # 32KB/part
            nc.gpsimd.dma_start(
                out=w0_sb, in_=w0_d[:, :].rearrange("(ko p) n -> p ko n", p=P))
            b0_bc = None
            if not trivial["b0"]:
                b0_bc = _bcast_load(nc, wres, vecs["b0"][:], D, "b0_bc")

            for st in range(SD):
                r1T = pool.tile([P, DD, P], F32R, tag="r1T")
                nc.sync.dma_start(
                    r1T, r1T_ds[st][:, :, :].rearrange("dk p s -> p dk s"))

                e_sb = pool.tile([P, D], FP32, tag="e")
                for dn in range(2):
                    ps = pp_e.tile([P, 512], FP32, tag="e", name="pse")
                    for k in range(DD):
                        _mm(nc, ps, r1T[:, k, :],
                            w0_sb[:, k, dn * 512:(dn + 1) * 512],
                            start=(k == 0), stop=(k == DD - 1))
                    dst = e_sb[:, dn * 512:(dn + 1) * 512]
                    nc.scalar.copy(dst, ps)
                    if b0_bc is not None:
                        nc.vector.tensor_add(
                            dst, dst, b0_bc[:, dn * 512:(dn + 1) * 512])
                nc.sync.dma_start(e_ds[st][:, :], e_sb)

                eT = pool.tile([P, DD, P], F32R, tag="eT")
                for dk in range(DD):
                    ps = pp_t.tile([P, P], FP32, tag="tr")
                    nc.tensor.transpose(ps, e_sb[:, dk * P:(dk + 1) * P], ident)
                    nc.vector.tensor_copy(eT[:, dk, :], ps)
                nc.sync.dma_start(
                    eT_ds[st][:, :, :].rearrange("dk p s -> p dk s"), eT)

        # -------- Phase C2: h, logits, out (w1 + w2 resident) --------
        with ExitStack() as pc2:
            wres = pc2.enter_context(tc.tile_pool(name="phC2_w", bufs=1))
            pool = pc2.enter_context(tc.tile_pool(name="phC2", bufs=2))
            pool1 = pc2.enter_context(tc.tile_pool(name="phC2_1", bufs=1))
            small = pc2.enter_context(tc.tile_pool(name="phC2_small", bufs=4))
            pp_h = pc2.enter_context(
                tc.tile_pool(name="ppC2_h", bufs=2, space="PSUM"))
            pp_l = pc2.enter_context(
                tc.tile_pool(name="ppC2_l", bufs=2, space="PSUM"))
            pp_t = pc2.enter_context(
                tc.tile_pool(name="ppC2_t", bufs=2, space="PSUM"))

            w1_sb = wres.tile([P, DD, H], F32R, tag="w1")   # 64KB/part
            nc.gpsimd.dma_start(
                out=w1_sb, in_=w1_d[:, :].rearrange("(ko p) n -> p ko n", p=P))
            w2_sb = wres.tile([P, HD, D], F32R, tag="w2")   # 64KB/part
            nc.gpsimd.dma_start(
                out=w2_sb, in_=w2_d[:, :].rearrange("(ko p) n -> p ko n", p=P))

            b1_bc = b2_bc = lng_bc = lnb_bc = n2g_bc = n2b_bc = None
            if not trivial["b1"]:
                b1_bc = _bcast_load(nc, wres, vecs["b1"][:], H, "b1_bc")
            if not trivial["b2"]:
                b2_bc = _bcast_load(nc, wres, vecs["b2"][:], D, "b2_bc")
            if not trivial["ln_g"]:
                lng_bc = _bcast_load(nc, wres, vecs["ln_g"][:], H, "lng_bc")
            if not trivial["ln_b"]:
                lnb_bc = _bcast_load(nc, wres, vecs["ln_b"][:], H, "lnb_bc")
            if not trivial["n2_g"]:
                n2g_bc = _bcast_load(nc, wres, vecs["n2_g"][:], D, "n2g_bc")
            if not trivial["n2_b"]:
                n2b_bc = _bcast_load(nc, wres, vecs["n2_b"][:], D, "n2b_bc")

            # colsum(w2) broadcast over partitions (fold path)
            w2s_bc = None
            if fold2:
                w2s_bc = wres.tile([P, D], FP32, tag="w2s")
                for dn in range(2):
                    ps = pp_l.tile([P, 512], FP32, tag="l", name="ps_w2s")
                    for k in range(HD):
                        _mm(nc, ps, ones_r, w2_sb[:, k, dn * 512:(dn + 1) * 512],
                            start=(k == 0), stop=(k == HD - 1))
                    nc.vector.tensor_copy(w2s_bc[:, dn * 512:(dn + 1) * 512], ps)

            for st in range(SD):
                eT = pool.tile([P, DD, P], F32R, tag="eT")
                nc.sync.dma_start(
                    eT, eT_ds[st][:, :, :].rearrange("dk p s -> p dk s"))
                e_sb = pool.tile([P, D], FP32, tag="e")
                nc.sync.dma_start(e_sb, e_ds[st][:, :])

                # h = lrelu(e @ w1 + b1)
                h_sb = pool.tile([P, H], FP32, tag="h")
                for hn in range(4):
                    ps = pp_h.tile([P, 512], FP32, tag="h", name="psh")
                    for k in range(DD):
                        _mm(nc, ps, eT[:, k, :],
                            w1_sb[:, k, hn * 512:(hn + 1) * 512],
                            start=(k == 0), stop=(k == DD - 1))
                    dst = h_sb[:, hn * 512:(hn + 1) * 512]
                    if b1_bc is not None:
                        nc.vector.tensor_add(
                            dst, ps, b1_bc[:, hn * 512:(hn + 1) * 512])
                        _lrelu(nc, dst, dst)
                    else:
                        _lrelu(nc, dst, ps)

                # LN2: stats only on the fold path
                ln2 = small.tile([P, 2], FP32, tag="ln2")
                _ln_stats(nc, small, ln2, h_sb, H, eps_sb)
                if fold2:
                    tr2_src = h_sb
                else:
                    h2 = pool.tile([P, H], FP32, tag="h2")
                    nc.vector.tensor_scalar(h2, h_sb, ln2[:, 0:1],
                                            ln2[:, 1:2], ALU.mult, ALU.add)
                    if lng_bc is not None:
                        nc.vector.tensor_mul(h2, h2, lng_bc)
                    if lnb_bc is not None:
                        nc.vector.tensor_add(h2, h2, lnb_bc)
                    tr2_src = h2

                # h -> hT (SBUF only, feeds the logits matmuls directly)
                hT = pool1.tile([P, HD, P], F32R, tag="hT")
                for hk in range(HD):
                    ps = pp_t.tile([P, P], FP32, tag="tr")
                    nc.tensor.transpose(
                        ps, tr2_src[:, hk * P:(hk + 1) * P], ident)
                    if hk % 2 == 0:
                        nc.vector.tensor_copy(hT[:, hk, :], ps)
                    else:
                        nc.scalar.copy(hT[:, hk, :], ps)

                # logits (+ fold2 LN2 affine) + e residual, lrelu, LN3
                t_sb = pool1.tile([P, D], FP32, tag="t")
                ltmp = None
                if fold2:
                    ltmp = pool1.tile([P, D], FP32, tag="ltmp")
                    nc.vector.tensor_scalar(ltmp, w2s_bc, ln2[:, 1:2], None,
                                            ALU.mult)
                    nc.vector.tensor_add(ltmp, ltmp, e_sb)
                    if b2_bc is not None:
                        nc.vector.tensor_add(ltmp, ltmp, b2_bc)
                for dn in range(2):
                    ps = pp_l.tile([P, 512], FP32, tag="l", name="psl")
                    for k in range(HD):
                        _mm(nc, ps, hT[:, k, :],
                            w2_sb[:, k, dn * 512:(dn + 1) * 512],
                            start=(k == 0), stop=(k == HD - 1))
                    dst = t_sb[:, dn * 512:(dn + 1) * 512]
                    if fold2:
                        nc.vector.scalar_tensor_tensor(
                            dst, ps, ln2[:, 0:1],
                            ltmp[:, dn * 512:(dn + 1) * 512],
                            op0=ALU.mult, op1=ALU.add)
                    else:
                        nc.vector.tensor_add(
                            dst, ps, e_sb[:, dn * 512:(dn + 1) * 512])
                        if b2_bc is not None:
                            nc.vector.tensor_add(
                                dst, dst, b2_bc[:, dn * 512:(dn + 1) * 512])
                _lrelu(nc, t_sb, t_sb)

                o_sb = pool.tile([P, D], FP32, tag="o")
                _layernorm(nc, small, o_sb, t_sb, D, eps_sb, n2g_bc, n2b_bc)
                nc.sync.dma_start(out_d[st * P:(st + 1) * P, :], o_sb)

    nc.compile()
    return nc


_CACHE = {}


def kernel(**inputs):
    x_emb = np.ascontiguousarray(inputs["x_embeddings"], dtype=np.float32)
    B = x_emb.shape[0]
    assert x_emb.shape == (B, S, D)

    trivial = {}
    for name in ["bq", "bk", "bv", "b0", "b1", "b2", "n1_b", "ln_b", "n2_b"]:
        trivial[name] = bool(np.all(np.asarray(inputs[name]) == 0.0))
    for name in ["n1_g", "ln_g", "n2_g"]:
        trivial[name] = bool(np.all(np.asarray(inputs[name]) == 1.0))

    key = tuple(sorted(trivial.items()))
    if key not in _CACHE:
        _CACHE[key] = build_kernel(trivial)
    nc = _CACHE[key]

    shared = {
        name: np.ascontiguousarray(inputs[name], dtype=np.float32)
        for name in ["wq", "wk", "wv", "w0", "w1", "w2"]
    }
    for name, triv in trivial.items():
        if not triv:
            shared[name] = np.ascontiguousarray(inputs[name], dtype=np.float32)

    in_maps = [dict(shared, x=x_emb[b]) for b in range(B)]
    res = run_bass_kernel_spmd(nc, in_maps, core_ids=list(range(N_CORES)))
    out = np.stack([res.results[b]["out"] for b in range(B)], axis=0)
    return out.astype(np.float32)
